# revision 20
# baseline (speedup 1.0000x reference)
"""GCMC graph-conv kernel for Trainium2, 8-core SPMD. v3.

out = ci * segment_sum((weight[node_ids] * cj)[src_idx], dst_idx)

Strategy (edge sharding by dst range, per-edge gather + one-hot matmul
segment-sum):
  - core k owns dst rows [k*12500, (k+1)*12500); host partitions edges and
    sorts by (tile_group, src_chunk, dst_tile, src)
  - the message table is fp16 with cj pre-folded, one row per src node,
    padded to a 256-byte row stride; SWDGE dma_gather fetches the 128-byte
    payload per edge (half the descriptor cost of a 256-byte fetch)
  - gathers are issued per (tile_group, chunk): ~50 large calls, amortizing
    the SWDGE per-call fixed overhead
  - slot segments are padded to 16 (not 128), so a 128-slot column can span
    two dst tiles; such boundary columns are processed once per tile with
    complementary masks (the second tile's dst values are offset by +128 and
    matched against a 128..255 iota)
  - DVE builds one-hot matrices oh[slot, d, col] = (iota[d] == dv[slot, col])
    in fp16, 16 columns per tensor_tensor; the batch dim is the minor axis so
    every operand AP keeps a packed last dim (2x DVE mode)
  - TensorE accumulates psum[tile] += oh_col.T @ msg_col per column (fp16,
    1 cycle/row); ACT writes psum*ci into a partition-major output buffer,
    flushed to DRAM once per group
"""
import sys, os
sys.path.insert(0, '/opt/trn_rl_repo')

import numpy as np

N_NODES = 100000
OUT_DIM = 64
N_CORES = 8
DST_PER_CORE = N_NODES // N_CORES          # 12500
N_CHUNKS = 4                                # int16 idx reach: 32767 rows/chunk
CHUNK = 25024                               # 4 * 25024 = 100096 table rows
TAB_ROWS = N_CHUNKS * CHUNK
TAB_STRIDE = 128                            # fp16 elems per row = 256 B
PAD_SENTINEL = 999.0
GROUP_TILES = 8                             # dst tiles per group (= psum banks)
OH_BATCH = 16                               # one-hot columns per DVE build
SEG_PAD = 16                                # slot-segment granularity


def _round_up(x, m):
    return (x + m - 1) // m * m


def _host_prep(src, dst):
    """Partition edges by dst range; build the shared (group, chunk) slot
    schedule with a cross-core envelope at (group, chunk) granularity.
    Per-core tile boundaries inside a call vary, so column->tile entries are
    the union over cores, with shared ranks (order of tile within column)."""
    n_tiles = _round_up(DST_PER_CORE, 128) // 128       # 98
    # 8-tile groups with a tapered tail: small trailing groups shorten the
    # post-gather compute tail
    sizes = [GROUP_TILES] * 11 + [4, 3, 2, 1]
    assert sum(sizes) == n_tiles

    raw = []
    counts = np.zeros((N_CORES, n_tiles, N_CHUNKS), np.int64)
    for k in range(N_CORES):
        m = (dst // DST_PER_CORE) == k
        s = src[m]
        dl = dst[m] - k * DST_PER_CORE
        counts[k] = np.bincount((dl // 128) * N_CHUNKS + s // CHUNK,
                                minlength=n_tiles * N_CHUNKS).reshape(
            n_tiles, N_CHUNKS)
        raw.append((s, dl))

    # greedy anti-correlated packing: assign tiles to groups minimizing the
    # cross-core max of per-(group, chunk) slot sums (the gather envelope)
    group_lists = [[] for _ in sizes]
    gsums = [np.zeros((N_CORES, N_CHUNKS)) for _ in sizes]
    for t in np.argsort(-counts.sum(axis=(0, 2))):
        best, best_d = -1, None
        for gi, sz in enumerate(sizes):
            if len(group_lists[gi]) >= sz:
                continue
            d = (np.maximum(gsums[gi] + counts[:, t, :], 0).max(axis=0).sum()
                 - gsums[gi].max(axis=0).sum())
            if best_d is None or d < best_d:
                best, best_d = gi, d
        group_lists[best].append(int(t))
        gsums[best] += counts[:, t, :]
    groups = [np.array(sorted(g)) for g in group_lists]
    n_groups = len(groups)
    t2g = np.zeros(n_tiles, np.int64)
    for gi, tiles_g in enumerate(groups):
        t2g[tiles_g] = gi

    per_core = []
    for k in range(N_CORES):
        s, dl = raw[k]
        t = dl // 128
        c = s // CHUNK
        order = np.lexsort((s, t, (t2g[t] * N_CHUNKS + c)))
        per_core.append((s[order], dl[order], t[order], c[order]))

    # (group, chunk) envelope, 128-aligned (= call size)
    gc_counts = np.stack([counts[:, g, :].sum(axis=1) for g in groups],
                         axis=1)                        # [cores, groups, chunks]
    env_gc = _round_up(gc_counts.max(axis=0), 128).astype(np.int64)

    calls = []          # (chunk, slot_off, n_idx)
    group_info = []     # (slot_base, n_cols, tiles)
    call_off = np.zeros((n_groups, N_CHUNKS), np.int64)
    off = 0
    for gi, tiles_g in enumerate(groups):
        g_base = off
        for c in range(N_CHUNKS):
            n = int(env_gc[gi, c])
            call_off[gi, c] = off
            if n:
                calls.append((c, off, n))
            off += n
        group_info.append((g_base, (off - g_base) // 128, tiles_g))
    total = off
    assert total % 128 == 0
    n_cols = total // 128

    # union column -> tiles over all cores' spans
    col_tiles = [set() for _ in range(n_cols)]
    span_cache = []     # per core: (slot0 per edge-group) for packing
    for k in range(N_CORES):
        for gi, tiles_g in enumerate(groups):
            for c in range(N_CHUNKS):
                a = int(call_off[gi, c])
                for t in tiles_g:
                    b = a + int(counts[k, t, c])
                    for col in range(a // 128, (b + 127) // 128):
                        if b > a:
                            col_tiles[col].add(int(t))
                    a = b

    col_entries = [[] for _ in range(n_cols)]
    rank1_tile = np.full(n_cols, -1, np.int64)
    for col in range(n_cols):
        ts = sorted(col_tiles[col])
        assert len(ts) <= 2, f"column {col} spans {len(ts)} tiles"
        col_entries[col] = [(t, r) for r, t in enumerate(ts)]
        if len(ts) == 2:
            rank1_tile[col] = ts[1]

    bnd_cols = [c for c in range(n_cols) if len(col_entries[c]) == 2]
    bnd_idx_of = {c: i for i, c in enumerate(bnd_cols)}

    tile_entries = [[] for _ in range(n_tiles)]
    for col, ents in enumerate(col_entries):
        for (t, rank) in ents:
            tile_entries[t].append((col, rank))
    for t in range(n_tiles):
        assert tile_entries[t], f"tile {t} empty"

    # per-core packed idx/dv
    idx_all, dv_all, dvb_all = [], [], []
    bnd = np.array(bnd_cols, np.int64)
    for k in range(N_CORES):
        s, dl, t, c = per_core[k]
        key = t2g[t] * N_CHUNKS + c
        key_counts = np.bincount(key, minlength=n_groups * N_CHUNKS)
        within = np.arange(len(s)) - np.repeat(
            np.concatenate([[0], np.cumsum(key_counts)])[:-1], key_counts)
        slot = call_off.reshape(-1)[key] + within
        idx_flat = np.zeros(total, np.int16)
        idx_flat[slot] = (s - c * CHUNK).astype(np.int16)
        dv_flat = np.full(total, PAD_SENTINEL, np.float32)
        dv_flat[slot] = (dl - t * 128) + 128 * (rank1_tile[slot // 128] == t)
        dv_flat = dv_flat.astype(np.float16)
        idx_all.append(np.tile(idx_flat.reshape(total // 16, 16).T, (8, 1)).copy())
        dv_w = np.ascontiguousarray(dv_flat.reshape(n_cols, 128).T)  # [128, cols]
        dv_all.append(np.ascontiguousarray(dv_w).reshape(128, 1, n_cols))
        dvb = dv_w[:, bnd] if len(bnd) else np.zeros((128, 1), np.float16)
        dvb_all.append(np.ascontiguousarray(dvb).reshape(128, 1, -1))
    return (total, calls, group_info, col_entries, tile_entries,
            bnd_idx_of, idx_all, dv_all, dvb_all)


def _raw_dma_gather(g, out_ap, in_ap, idxs_ap, num_idxs, elem_size, elem_step):
    """dma_gather without the elem_size_bytes % 256 restriction (which only
    applies to transpose mode); descriptor stride must still be a multiple
    of 256 bytes. Mirrors bass.BassEngine.dma_gather's non-transpose path."""
    import concourse.mybir as mybir
    import concourse.ap_utils as ap_utils
    from concourse.bass import MemorySpace, round_up_to_multiple, exact_div
    assert idxs_ap.dtype == mybir.dt.int16
    assert in_ap.dtype == out_ap.dtype
    assert in_ap.space == MemorySpace.DRAM
    assert idxs_ap.space == MemorySpace.SBUF and out_ap.space == MemorySpace.SBUF
    assert ap_utils.ap_is_contiguous(out_ap.ap[1:])
    assert ap_utils.ap_is_contiguous(idxs_ap.ap[1:])
    assert in_ap.ap[-1][1] == out_ap.ap[-1][1] == elem_size
    assert out_ap.ap[0][1] * out_ap.ap[1][1] == round_up_to_multiple(num_idxs, 128)
    assert in_ap.ap[0][0] == elem_step
    stride_bytes_256 = exact_div(elem_step * mybir.dt.size(in_ap.dtype), 256)
    assert stride_bytes_256 < 256
    _in_ap = g.lower_ap_dma(in_ap, for_custom_bir_dma=True)
    return g.add_instruction(
        mybir.InstDMAGatherAnt(
            name=g.bass.get_next_instruction_name(),
            ins=[*_in_ap, g.lower_ap(idxs_ap),
                 g.lower_val_access(g.to_reg(num_idxs))],
            outs=[g.lower_ap(out_ap)],
            transpose=False,
            num_idxs=num_idxs,
            elem_size=elem_size,
            stride_bytes_256=stride_bytes_256,
            gen_mode=0,
            single_packet=False,
            queue_num=0,
            sbuf_tokens_per_rank=0,
            sbuf_free_dim_per_rank=0,
            sbuf_free_dim_pad_per_rank=0,
            sbuf_byte_offset=0,
        ))


def _build_program(total, calls, group_info, col_entries, tile_entries,
                   bnd_idx_of):
    import concourse.bacc as bacc
    import concourse.mybir as mybir
    import concourse.tile as tile

    n_tiles = len(tile_entries)
    n_cols = total // 128
    n_bnd = max(1, len(bnd_idx_of))
    f32 = mybir.dt.float32
    f16 = mybir.dt.float16

    nc = bacc.Bacc("TRN2", target_bir_lowering=False, debug=False,
                   num_devices=N_CORES)
    tab_d = nc.dram_tensor("tab", [TAB_ROWS, TAB_STRIDE], f16,
                           kind="ExternalInput").ap()
    ci_d = nc.dram_tensor("ci", [128, n_tiles], f32, kind="ExternalInput").ap()
    io_d = nc.dram_tensor("io", [128, 128, 2 * OH_BATCH], f16,
                          kind="ExternalInput").ap()
    idx_d = nc.dram_tensor("idx", [128, total // 16], mybir.dt.int16,
                           kind="ExternalInput").ap()
    dv_d = nc.dram_tensor("dv", [128, 1, n_cols], f16,
                          kind="ExternalInput").ap()
    dvb_d = nc.dram_tensor("dvb", [128, 1, n_bnd], f16,
                           kind="ExternalInput").ap()
    out_d = nc.dram_tensor("out", [128, n_tiles * OUT_DIM], f32,
                           kind="ExternalOutput").ap()

    first = {t: ents[0] for t, ents in enumerate(tile_entries)}
    last = {t: ents[-1] for t, ents in enumerate(tile_entries)}
    pos_of, p = {}, 0
    for (_, _, tiles_g) in group_info:
        for t in tiles_g:
            pos_of[int(t)] = p
            p += 1

    with tile.TileContext(nc) as tc:
        with (
            tc.tile_pool(name="const", bufs=1) as constp,
            tc.tile_pool(name="msg", bufs=2) as msgp,
            tc.tile_pool(name="oh", bufs=3) as ohp,
            tc.tile_pool(name="ohb", bufs=2) as ohbp,
            tc.tile_pool(name="ps", bufs=8, space="PSUM") as psp,
        ):
            ci_t = constp.tile([128, n_tiles], f32)
            io_t = constp.tile([128, 128, 2 * OH_BATCH], f16)
            idx_t = constp.tile([128, total // 16], mybir.dt.int16)
            dv_t = constp.tile([128, 1, n_cols], f16)
            dvb_t = constp.tile([128, 1, n_bnd], f16)
            outb_t = constp.tile([128, n_tiles * OUT_DIM], f32)
            # chunk metadata uploads so early groups only wait on their
            # slice; group 0's idx goes first so the first gather starts
            # without queueing behind the 1MB iota constant
            for gi, (g_base, g_cols, _) in enumerate(group_info):
                a16, b16 = g_base // 16, (g_base + g_cols * 128) // 16
                nc.sync.dma_start(idx_t[:, a16:b16], idx_d[:, a16:b16])
                a, b = g_base // 128, g_base // 128 + g_cols
                nc.sync.dma_start(dv_t[:, :, a:b], dv_d[:, :, a:b])
                if gi == 0:
                    nc.sync.dma_start(ci_t[:], ci_d[:])
                    nc.sync.dma_start(io_t[:], io_d[:])
                    nc.sync.dma_start(dvb_t[:], dvb_d[:])

            call_i = 0
            for gi, (g_base, g_cols, tiles_g) in enumerate(group_info):
                g_col0 = g_base // 128
                msg_t = msgp.tile([128, g_cols, OUT_DIM], f16, tag="msg")
                # gathers: one per chunk for this group's slot range
                while call_i < len(calls) and \
                        g_base <= calls[call_i][1] < g_base + g_cols * 128:
                    c, off, n = calls[call_i]
                    lc = (off - g_base) // 128
                    _raw_dma_gather(
                        nc.gpsimd,
                        msg_t[:, lc:lc + n // 128, :],
                        tab_d[c * CHUNK:(c + 1) * CHUNK, 0:OUT_DIM],
                        idx_t[:, off // 16:(off + n) // 16],
                        n, OUT_DIM, TAB_STRIDE)
                    call_i += 1
                # one-hot builds: OH_BATCH columns per tensor_tensor, batch
                # dim minor so every AP keeps a packed last dim (2x mode)
                oh_of = {}
                for a in range(0, g_cols, OH_BATCH):
                    nb = min(OH_BATCH, g_cols - a)
                    oh_t = ohp.tile([128, 128, nb], f16, tag="oh")
                    nc.vector.tensor_tensor(
                        oh_t[:],
                        io_t[:, :, 0:nb],
                        dv_t[:, :, g_col0 + a:g_col0 + a + nb]
                            .to_broadcast([128, 128, nb]),
                        mybir.AluOpType.is_equal)
                    for j in range(nb):
                        oh_of[g_col0 + a + j] = (oh_t, j)
                # alt builds for this group's boundary columns (rank-1 masks,
                # iota offset by +128)
                g_bnd = [c for c in range(g_col0, g_col0 + g_cols)
                         if c in bnd_idx_of]
                ohb_of = {}
                for a in range(0, len(g_bnd), OH_BATCH):
                    nb = min(OH_BATCH, len(g_bnd) - a)
                    b0 = bnd_idx_of[g_bnd[a]]
                    oh_t = ohbp.tile([128, 128, nb], f16, tag="ohb")
                    nc.vector.tensor_tensor(
                        oh_t[:],
                        io_t[:, :, OH_BATCH:OH_BATCH + nb],
                        dvb_t[:, :, b0:b0 + nb].to_broadcast([128, 128, nb]),
                        mybir.AluOpType.is_equal)
                    for j in range(nb):
                        ohb_of[g_bnd[a + j]] = (oh_t, j)
                # column-major matmuls; one 2KB psum bank per tile chain
                ps_of = {int(t): psp.tile([128, OUT_DIM], f32, tag="ps",
                                          name=f"ps{gi}_{j}")[:]
                         for j, t in enumerate(tiles_g)}
                for col in range(g_col0, g_col0 + g_cols):
                    for (t, rank) in col_entries[col]:
                        oh_t, j = oh_of[col] if rank == 0 else ohb_of[col]
                        nc.tensor.matmul(ps_of[t], oh_t[:, :, j],
                                         msg_t[:, col - g_col0, :],
                                         start=((col, rank) == first[t]),
                                         stop=((col, rank) == last[t]))
                        if (col, rank) == last[t]:
                            pp = pos_of[t]
                            nc.scalar.activation(
                                outb_t[:, pp * OUT_DIM:(pp + 1) * OUT_DIM],
                                ps_of[t],
                                mybir.ActivationFunctionType.Copy,
                                scale=ci_t[:, t:t + 1])
                p0 = pos_of[int(tiles_g[0])]
                p1 = p0 + len(tiles_g)
                nc.sync.dma_start(
                    out_d[:, p0 * OUT_DIM:p1 * OUT_DIM],
                    outb_t[:, p0 * OUT_DIM:p1 * OUT_DIM])

    nc.compile()
    return nc


def prepare(node_ids, src_idx, dst_idx, cj, ci, weight):
    """Host prep + program build. Returns (nc, in_maps, postprocess)."""
    import time
    _t0 = time.time()

    node_ids = np.asarray(node_ids)
    src = np.asarray(src_idx).astype(np.int64)
    dst = np.asarray(dst_idx).astype(np.int64)
    cj = np.asarray(cj, dtype=np.float32).reshape(-1)
    ci = np.asarray(ci, dtype=np.float32).reshape(-1)
    weight = np.asarray(weight, dtype=np.float32)

    # feat rows are weight[node_ids]; with the arange fill this is identity
    if not np.array_equal(node_ids, np.arange(N_NODES, dtype=node_ids.dtype)):
        weight = weight[node_ids]

    tab = np.zeros((TAB_ROWS, TAB_STRIDE), np.float16)
    tab[:N_NODES, :OUT_DIM] = (weight * cj[:, None]).astype(np.float16)

    n_tiles = _round_up(DST_PER_CORE, 128) // 128
    # io[p, i, b] = i for b < OH_BATCH (rank-0), 128+i for b >= OH_BATCH
    iota = np.arange(128, dtype=np.float32)
    io = np.empty((128, 128, 2 * OH_BATCH), np.float16)
    io[:, :, :OH_BATCH] = iota[None, :, None].astype(np.float16)
    io[:, :, OH_BATCH:] = (iota + 128)[None, :, None].astype(np.float16)

    (total, calls, group_info, col_entries, tile_entries, bnd_idx_of,
     idx_all, dv_all, dvb_all) = _host_prep(src, dst)
    print(f"[kernel] host prep: {time.time()-_t0:.1f}s (total slots {total}, "
          f"bnd cols {len(bnd_idx_of)})", flush=True)
    _t1 = time.time()
    nc = _build_program(total, calls, group_info, col_entries, tile_entries,
                        bnd_idx_of)
    print(f"[kernel] build+schedule+compile-to-bir: {time.time()-_t1:.1f}s",
          flush=True)

    in_maps = []
    for k in range(N_CORES):
        ci_k = np.zeros(n_tiles * 128, np.float32)
        ci_k[:DST_PER_CORE] = ci[k * DST_PER_CORE:(k + 1) * DST_PER_CORE]
        ci_w = ci_k.reshape(n_tiles, 128).T.copy()
        in_maps.append({
            "tab": tab, "ci": ci_w, "io": io,
            "idx": idx_all[k], "dv": dv_all[k], "dvb": dvb_all[k],
        })

    tile_at_pos = np.concatenate([g for (_, _, g) in
                                  [gi for gi in group_info]]) \
        if False else np.concatenate([g for (_, _, g) in group_info])

    def post(results):
        # out is partition- and position-major [128, n_tiles*64]: dst row
        # t*128+p is at out[p, pos_of[t]*64:(pos_of[t]+1)*64]
        outs = []
        for k in range(N_CORES):
            posarr = results[k]["out"].reshape(128, n_tiles, OUT_DIM) \
                .transpose(1, 0, 2)
            bytile = np.empty_like(posarr)
            bytile[tile_at_pos] = posarr
            outs.append(bytile.reshape(n_tiles * 128, OUT_DIM)[:DST_PER_CORE])
        return np.concatenate(outs, axis=0)

    return nc, in_maps, post


def kernel(node_ids, src_idx, dst_idx, cj, ci, weight):
    import time
    from concourse.bass_utils import run_bass_kernel_spmd
    nc, in_maps, post = prepare(node_ids, src_idx, dst_idx, cj, ci, weight)
    _t2 = time.time()
    try:
        res = run_bass_kernel_spmd(nc, in_maps, core_ids=list(range(N_CORES)))
    except Exception as e:
        # transient device wedges (NRT_EXEC_UNIT_UNRECOVERABLE) clear on retry
        print(f"[kernel] exec failed ({type(e).__name__}), retrying", flush=True)
        time.sleep(5)
        res = run_bass_kernel_spmd(nc, in_maps, core_ids=list(range(N_CORES)))
    print(f"[kernel] neff compile+exec: {time.time()-_t2:.1f}s", flush=True)
    return post(res.results)


# revision 21
# speedup vs baseline: 1.0030x; 1.0030x over previous
"""GCMC graph-conv kernel for Trainium2, 8-core SPMD. v3.

out = ci * segment_sum((weight[node_ids] * cj)[src_idx], dst_idx)

Strategy (edge sharding by dst range, per-edge gather + one-hot matmul
segment-sum):
  - core k owns dst rows [k*12500, (k+1)*12500); host partitions edges and
    sorts by (tile_group, src_chunk, dst_tile, src)
  - the message table is fp16 with cj pre-folded, one row per src node,
    padded to a 256-byte row stride; SWDGE dma_gather fetches the 128-byte
    payload per edge (half the descriptor cost of a 256-byte fetch)
  - gathers are issued per (tile_group, chunk): ~50 large calls, amortizing
    the SWDGE per-call fixed overhead
  - slot segments are padded to 16 (not 128), so a 128-slot column can span
    two dst tiles; such boundary columns are processed once per tile with
    complementary masks (the second tile's dst values are offset by +128 and
    matched against a 128..255 iota)
  - DVE builds one-hot matrices oh[slot, d, col] = (iota[d] == dv[slot, col])
    in fp16, 16 columns per tensor_tensor; the batch dim is the minor axis so
    every operand AP keeps a packed last dim (2x DVE mode)
  - TensorE accumulates psum[tile] += oh_col.T @ msg_col per column (fp16,
    1 cycle/row); ACT writes psum*ci into a partition-major output buffer,
    flushed to DRAM once per group
"""
import sys, os
sys.path.insert(0, '/opt/trn_rl_repo')

import numpy as np

N_NODES = 100000
OUT_DIM = 64
N_CORES = 8
DST_PER_CORE = N_NODES // N_CORES          # 12500
N_CHUNKS = 4                                # int16 idx reach: 32767 rows/chunk
CHUNK = 25024                               # 4 * 25024 = 100096 table rows
TAB_ROWS = N_CHUNKS * CHUNK
TAB_STRIDE = 128                            # fp16 elems per row = 256 B
PAD_SENTINEL = 999.0
GROUP_TILES = 8                             # dst tiles per group (= psum banks)
OH_BATCH = 16                               # one-hot columns per DVE build
SEG_PAD = 16                                # slot-segment granularity


def _round_up(x, m):
    return (x + m - 1) // m * m


def _host_prep(src, dst):
    """Partition edges by dst range; build the shared (group, chunk) slot
    schedule with a cross-core envelope at (group, chunk) granularity.
    Per-core tile boundaries inside a call vary, so column->tile entries are
    the union over cores, with shared ranks (order of tile within column)."""
    n_tiles = _round_up(DST_PER_CORE, 128) // 128       # 98
    # 8-tile groups with a tapered tail: small trailing groups shorten the
    # post-gather compute tail
    sizes = [GROUP_TILES] * 11 + [4, 3, 2, 1]
    assert sum(sizes) == n_tiles

    raw = []
    counts = np.zeros((N_CORES, n_tiles, N_CHUNKS), np.int64)
    for k in range(N_CORES):
        m = (dst // DST_PER_CORE) == k
        s = src[m]
        dl = dst[m] - k * DST_PER_CORE
        counts[k] = np.bincount((dl // 128) * N_CHUNKS + s // CHUNK,
                                minlength=n_tiles * N_CHUNKS).reshape(
            n_tiles, N_CHUNKS)
        raw.append((s, dl))

    # greedy anti-correlated packing: assign tiles to groups minimizing the
    # cross-core max of per-(group, chunk) slot sums (the gather envelope)
    group_lists = [[] for _ in sizes]
    gsums = [np.zeros((N_CORES, N_CHUNKS)) for _ in sizes]
    for t in np.argsort(-counts.sum(axis=(0, 2))):
        best, best_d = -1, None
        for gi, sz in enumerate(sizes):
            if len(group_lists[gi]) >= sz:
                continue
            d = (np.maximum(gsums[gi] + counts[:, t, :], 0).max(axis=0).sum()
                 - gsums[gi].max(axis=0).sum())
            if best_d is None or d < best_d:
                best, best_d = gi, d
        group_lists[best].append(int(t))
        gsums[best] += counts[:, t, :]
    groups = [np.array(sorted(g)) for g in group_lists]
    n_groups = len(groups)
    t2g = np.zeros(n_tiles, np.int64)
    for gi, tiles_g in enumerate(groups):
        t2g[tiles_g] = gi

    per_core = []
    for k in range(N_CORES):
        s, dl = raw[k]
        t = dl // 128
        c = s // CHUNK
        order = np.lexsort((s, t, (t2g[t] * N_CHUNKS + c)))
        per_core.append((s[order], dl[order], t[order], c[order]))

    # (group, chunk) envelope: descriptor counts are 16-granular (the cost
    # is per descriptor), but each call's slot region is 128-aligned so the
    # next call starts on a column boundary; the un-gathered tail of a
    # call's last column is memset to zero in the program
    gc_counts = np.stack([counts[:, g, :].sum(axis=1) for g in groups],
                         axis=1)                        # [cores, groups, chunks]
    env_gc = _round_up(gc_counts.max(axis=0), SEG_PAD).astype(np.int64)

    calls = []          # (chunk, slot_off, n_idx % 16 == 0)
    group_info = []     # (slot_base, n_cols, tiles)
    call_off = np.zeros((n_groups, N_CHUNKS), np.int64)
    off = 0
    for gi, tiles_g in enumerate(groups):
        g_base = off
        for c in range(N_CHUNKS):
            n = int(env_gc[gi, c])
            call_off[gi, c] = off
            if n:
                calls.append((c, off, n))
            off += _round_up(n, 128)
        group_info.append((g_base, (off - g_base) // 128, tiles_g))
    total = off
    assert total % 128 == 0
    n_cols = total // 128

    # union column -> tiles over all cores' spans
    col_tiles = [set() for _ in range(n_cols)]
    span_cache = []     # per core: (slot0 per edge-group) for packing
    for k in range(N_CORES):
        for gi, tiles_g in enumerate(groups):
            for c in range(N_CHUNKS):
                a = int(call_off[gi, c])
                for t in tiles_g:
                    b = a + int(counts[k, t, c])
                    for col in range(a // 128, (b + 127) // 128):
                        if b > a:
                            col_tiles[col].add(int(t))
                    a = b

    col_entries = [[] for _ in range(n_cols)]
    rank1_tile = np.full(n_cols, -1, np.int64)
    for col in range(n_cols):
        ts = sorted(col_tiles[col])
        assert len(ts) <= 2, f"column {col} spans {len(ts)} tiles"
        col_entries[col] = [(t, r) for r, t in enumerate(ts)]
        if len(ts) == 2:
            rank1_tile[col] = ts[1]

    bnd_cols = [c for c in range(n_cols) if len(col_entries[c]) == 2]
    bnd_idx_of = {c: i for i, c in enumerate(bnd_cols)}

    tile_entries = [[] for _ in range(n_tiles)]
    for col, ents in enumerate(col_entries):
        for (t, rank) in ents:
            tile_entries[t].append((col, rank))
    for t in range(n_tiles):
        assert tile_entries[t], f"tile {t} empty"

    # per-core packed idx/dv
    idx_all, dv_all, dvb_all = [], [], []
    bnd = np.array(bnd_cols, np.int64)
    for k in range(N_CORES):
        s, dl, t, c = per_core[k]
        key = t2g[t] * N_CHUNKS + c
        key_counts = np.bincount(key, minlength=n_groups * N_CHUNKS)
        within = np.arange(len(s)) - np.repeat(
            np.concatenate([[0], np.cumsum(key_counts)])[:-1], key_counts)
        slot = call_off.reshape(-1)[key] + within
        idx_flat = np.zeros(total, np.int16)
        idx_flat[slot] = (s - c * CHUNK).astype(np.int16)
        dv_flat = np.full(total, PAD_SENTINEL, np.float32)
        dv_flat[slot] = (dl - t * 128) + 128 * (rank1_tile[slot // 128] == t)
        dv_flat = dv_flat.astype(np.float16)
        idx_all.append(np.tile(idx_flat.reshape(total // 16, 16).T, (8, 1)).copy())
        dv_w = np.ascontiguousarray(dv_flat.reshape(n_cols, 128).T)  # [128, cols]
        dv_all.append(np.ascontiguousarray(dv_w).reshape(128, 1, n_cols))
        dvb = dv_w[:, bnd] if len(bnd) else np.zeros((128, 1), np.float16)
        dvb_all.append(np.ascontiguousarray(dvb).reshape(128, 1, -1))
    return (total, calls, group_info, col_entries, tile_entries,
            bnd_idx_of, idx_all, dv_all, dvb_all)


def _raw_dma_gather(g, out_ap, in_ap, idxs_ap, num_idxs, elem_size, elem_step):
    """dma_gather without the elem_size_bytes % 256 restriction (which only
    applies to transpose mode); descriptor stride must still be a multiple
    of 256 bytes. Mirrors bass.BassEngine.dma_gather's non-transpose path."""
    import concourse.mybir as mybir
    import concourse.ap_utils as ap_utils
    from concourse.bass import MemorySpace, round_up_to_multiple, exact_div
    assert idxs_ap.dtype == mybir.dt.int16
    assert in_ap.dtype == out_ap.dtype
    assert in_ap.space == MemorySpace.DRAM
    assert idxs_ap.space == MemorySpace.SBUF and out_ap.space == MemorySpace.SBUF
    assert ap_utils.ap_is_contiguous(out_ap.ap[1:])
    assert ap_utils.ap_is_contiguous(idxs_ap.ap[1:])
    assert in_ap.ap[-1][1] == out_ap.ap[-1][1] == elem_size
    assert out_ap.ap[0][1] * out_ap.ap[1][1] == round_up_to_multiple(num_idxs, 128)
    assert in_ap.ap[0][0] == elem_step
    stride_bytes_256 = exact_div(elem_step * mybir.dt.size(in_ap.dtype), 256)
    assert stride_bytes_256 < 256
    _in_ap = g.lower_ap_dma(in_ap, for_custom_bir_dma=True)
    return g.add_instruction(
        mybir.InstDMAGatherAnt(
            name=g.bass.get_next_instruction_name(),
            ins=[*_in_ap, g.lower_ap(idxs_ap),
                 g.lower_val_access(g.to_reg(num_idxs))],
            outs=[g.lower_ap(out_ap)],
            transpose=False,
            num_idxs=num_idxs,
            elem_size=elem_size,
            stride_bytes_256=stride_bytes_256,
            gen_mode=0,
            single_packet=False,
            queue_num=0,
            sbuf_tokens_per_rank=0,
            sbuf_free_dim_per_rank=0,
            sbuf_free_dim_pad_per_rank=0,
            sbuf_byte_offset=0,
        ))


def _build_program(total, calls, group_info, col_entries, tile_entries,
                   bnd_idx_of):
    import concourse.bacc as bacc
    import concourse.mybir as mybir
    import concourse.tile as tile

    n_tiles = len(tile_entries)
    n_cols = total // 128
    n_bnd = max(1, len(bnd_idx_of))
    f32 = mybir.dt.float32
    f16 = mybir.dt.float16

    nc = bacc.Bacc("TRN2", target_bir_lowering=False, debug=False,
                   num_devices=N_CORES)
    tab_d = nc.dram_tensor("tab", [TAB_ROWS, TAB_STRIDE], f16,
                           kind="ExternalInput").ap()
    ci_d = nc.dram_tensor("ci", [128, n_tiles], f32, kind="ExternalInput").ap()
    io_d = nc.dram_tensor("io", [128, 128, 2 * OH_BATCH], f16,
                          kind="ExternalInput").ap()
    idx_d = nc.dram_tensor("idx", [128, total // 16], mybir.dt.int16,
                           kind="ExternalInput").ap()
    dv_d = nc.dram_tensor("dv", [128, 1, n_cols], f16,
                          kind="ExternalInput").ap()
    dvb_d = nc.dram_tensor("dvb", [128, 1, n_bnd], f16,
                           kind="ExternalInput").ap()
    out_d = nc.dram_tensor("out", [128, n_tiles * OUT_DIM], f32,
                           kind="ExternalOutput").ap()

    first = {t: ents[0] for t, ents in enumerate(tile_entries)}
    last = {t: ents[-1] for t, ents in enumerate(tile_entries)}
    pos_of, p = {}, 0
    for (_, _, tiles_g) in group_info:
        for t in tiles_g:
            pos_of[int(t)] = p
            p += 1

    with tile.TileContext(nc) as tc:
        with (
            tc.tile_pool(name="const", bufs=1) as constp,
            tc.tile_pool(name="msg", bufs=2) as msgp,
            tc.tile_pool(name="oh", bufs=3) as ohp,
            tc.tile_pool(name="ohb", bufs=2) as ohbp,
            tc.tile_pool(name="ps", bufs=8, space="PSUM") as psp,
        ):
            ci_t = constp.tile([128, n_tiles], f32)
            io_t = constp.tile([128, 128, 2 * OH_BATCH], f16)
            idx_t = constp.tile([128, total // 16], mybir.dt.int16)
            dv_t = constp.tile([128, 1, n_cols], f16)
            dvb_t = constp.tile([128, 1, n_bnd], f16)
            outb_t = constp.tile([128, n_tiles * OUT_DIM], f32)
            # chunk metadata uploads so early groups only wait on their
            # slice; group 0's idx goes first so the first gather starts
            # without queueing behind the 1MB iota constant
            for gi, (g_base, g_cols, _) in enumerate(group_info):
                a16, b16 = g_base // 16, (g_base + g_cols * 128) // 16
                nc.sync.dma_start(idx_t[:, a16:b16], idx_d[:, a16:b16])
                a, b = g_base // 128, g_base // 128 + g_cols
                nc.sync.dma_start(dv_t[:, :, a:b], dv_d[:, :, a:b])
                if gi == 0:
                    nc.sync.dma_start(ci_t[:], ci_d[:])
                    nc.sync.dma_start(io_t[:], io_d[:])
                    nc.sync.dma_start(dvb_t[:], dvb_d[:])

            call_i = 0
            for gi, (g_base, g_cols, tiles_g) in enumerate(group_info):
                g_col0 = g_base // 128
                msg_t = msgp.tile([128, g_cols, OUT_DIM], f16, tag="msg")
                # gathers: one per chunk for this group's slot range
                while call_i < len(calls) and \
                        g_base <= calls[call_i][1] < g_base + g_cols * 128:
                    c, off, n = calls[call_i]
                    lc = (off - g_base) // 128
                    ncols_c = (n + 127) // 128
                    if n % 128:
                        # zero the partial last column: its tail slots are
                        # not gathered and must not feed junk to the matmul
                        nc.gpsimd.memset(
                            msg_t[:, lc + ncols_c - 1:lc + ncols_c, :], 0)
                    _raw_dma_gather(
                        nc.gpsimd,
                        msg_t[:, lc:lc + ncols_c, :],
                        tab_d[c * CHUNK:(c + 1) * CHUNK, 0:OUT_DIM],
                        idx_t[:, off // 16:(off + n) // 16],
                        n, OUT_DIM, TAB_STRIDE)
                    call_i += 1
                # one-hot builds: OH_BATCH columns per tensor_tensor, batch
                # dim minor so every AP keeps a packed last dim (2x mode)
                oh_of = {}
                for a in range(0, g_cols, OH_BATCH):
                    nb = min(OH_BATCH, g_cols - a)
                    oh_t = ohp.tile([128, 128, nb], f16, tag="oh")
                    nc.vector.tensor_tensor(
                        oh_t[:],
                        io_t[:, :, 0:nb],
                        dv_t[:, :, g_col0 + a:g_col0 + a + nb]
                            .to_broadcast([128, 128, nb]),
                        mybir.AluOpType.is_equal)
                    for j in range(nb):
                        oh_of[g_col0 + a + j] = (oh_t, j)
                # alt builds for this group's boundary columns (rank-1 masks,
                # iota offset by +128)
                g_bnd = [c for c in range(g_col0, g_col0 + g_cols)
                         if c in bnd_idx_of]
                ohb_of = {}
                for a in range(0, len(g_bnd), OH_BATCH):
                    nb = min(OH_BATCH, len(g_bnd) - a)
                    b0 = bnd_idx_of[g_bnd[a]]
                    oh_t = ohbp.tile([128, 128, nb], f16, tag="ohb")
                    nc.vector.tensor_tensor(
                        oh_t[:],
                        io_t[:, :, OH_BATCH:OH_BATCH + nb],
                        dvb_t[:, :, b0:b0 + nb].to_broadcast([128, 128, nb]),
                        mybir.AluOpType.is_equal)
                    for j in range(nb):
                        ohb_of[g_bnd[a + j]] = (oh_t, j)
                # column-major matmuls; one 2KB psum bank per tile chain
                ps_of = {int(t): psp.tile([128, OUT_DIM], f32, tag="ps",
                                          name=f"ps{gi}_{j}")[:]
                         for j, t in enumerate(tiles_g)}
                for col in range(g_col0, g_col0 + g_cols):
                    for (t, rank) in col_entries[col]:
                        oh_t, j = oh_of[col] if rank == 0 else ohb_of[col]
                        nc.tensor.matmul(ps_of[t], oh_t[:, :, j],
                                         msg_t[:, col - g_col0, :],
                                         start=((col, rank) == first[t]),
                                         stop=((col, rank) == last[t]))
                        if (col, rank) == last[t]:
                            pp = pos_of[t]
                            nc.scalar.activation(
                                outb_t[:, pp * OUT_DIM:(pp + 1) * OUT_DIM],
                                ps_of[t],
                                mybir.ActivationFunctionType.Copy,
                                scale=ci_t[:, t:t + 1])
                p0 = pos_of[int(tiles_g[0])]
                p1 = p0 + len(tiles_g)
                nc.sync.dma_start(
                    out_d[:, p0 * OUT_DIM:p1 * OUT_DIM],
                    outb_t[:, p0 * OUT_DIM:p1 * OUT_DIM])

    nc.compile()
    return nc


def prepare(node_ids, src_idx, dst_idx, cj, ci, weight):
    """Host prep + program build. Returns (nc, in_maps, postprocess)."""
    import time
    _t0 = time.time()

    node_ids = np.asarray(node_ids)
    src = np.asarray(src_idx).astype(np.int64)
    dst = np.asarray(dst_idx).astype(np.int64)
    cj = np.asarray(cj, dtype=np.float32).reshape(-1)
    ci = np.asarray(ci, dtype=np.float32).reshape(-1)
    weight = np.asarray(weight, dtype=np.float32)

    # feat rows are weight[node_ids]; with the arange fill this is identity
    if not np.array_equal(node_ids, np.arange(N_NODES, dtype=node_ids.dtype)):
        weight = weight[node_ids]

    tab = np.zeros((TAB_ROWS, TAB_STRIDE), np.float16)
    tab[:N_NODES, :OUT_DIM] = (weight * cj[:, None]).astype(np.float16)

    n_tiles = _round_up(DST_PER_CORE, 128) // 128
    # io[p, i, b] = i for b < OH_BATCH (rank-0), 128+i for b >= OH_BATCH
    iota = np.arange(128, dtype=np.float32)
    io = np.empty((128, 128, 2 * OH_BATCH), np.float16)
    io[:, :, :OH_BATCH] = iota[None, :, None].astype(np.float16)
    io[:, :, OH_BATCH:] = (iota + 128)[None, :, None].astype(np.float16)

    (total, calls, group_info, col_entries, tile_entries, bnd_idx_of,
     idx_all, dv_all, dvb_all) = _host_prep(src, dst)
    print(f"[kernel] host prep: {time.time()-_t0:.1f}s (total slots {total}, "
          f"bnd cols {len(bnd_idx_of)})", flush=True)
    _t1 = time.time()
    nc = _build_program(total, calls, group_info, col_entries, tile_entries,
                        bnd_idx_of)
    print(f"[kernel] build+schedule+compile-to-bir: {time.time()-_t1:.1f}s",
          flush=True)

    in_maps = []
    for k in range(N_CORES):
        ci_k = np.zeros(n_tiles * 128, np.float32)
        ci_k[:DST_PER_CORE] = ci[k * DST_PER_CORE:(k + 1) * DST_PER_CORE]
        ci_w = ci_k.reshape(n_tiles, 128).T.copy()
        in_maps.append({
            "tab": tab, "ci": ci_w, "io": io,
            "idx": idx_all[k], "dv": dv_all[k], "dvb": dvb_all[k],
        })

    tile_at_pos = np.concatenate([g for (_, _, g) in
                                  [gi for gi in group_info]]) \
        if False else np.concatenate([g for (_, _, g) in group_info])

    def post(results):
        # out is partition- and position-major [128, n_tiles*64]: dst row
        # t*128+p is at out[p, pos_of[t]*64:(pos_of[t]+1)*64]
        outs = []
        for k in range(N_CORES):
            posarr = results[k]["out"].reshape(128, n_tiles, OUT_DIM) \
                .transpose(1, 0, 2)
            bytile = np.empty_like(posarr)
            bytile[tile_at_pos] = posarr
            outs.append(bytile.reshape(n_tiles * 128, OUT_DIM)[:DST_PER_CORE])
        return np.concatenate(outs, axis=0)

    return nc, in_maps, post


def kernel(node_ids, src_idx, dst_idx, cj, ci, weight):
    import time
    from concourse.bass_utils import run_bass_kernel_spmd
    nc, in_maps, post = prepare(node_ids, src_idx, dst_idx, cj, ci, weight)
    _t2 = time.time()
    try:
        res = run_bass_kernel_spmd(nc, in_maps, core_ids=list(range(N_CORES)))
    except Exception as e:
        # transient device wedges (NRT_EXEC_UNIT_UNRECOVERABLE) clear on retry
        print(f"[kernel] exec failed ({type(e).__name__}), retrying", flush=True)
        time.sleep(5)
        res = run_bass_kernel_spmd(nc, in_maps, core_ids=list(range(N_CORES)))
    print(f"[kernel] neff compile+exec: {time.time()-_t2:.1f}s", flush=True)
    return post(res.results)
